# revision 1
# baseline (speedup 1.0000x reference)
"""DLRM (26-table embedding + pairwise interaction + MLPs) on 8 Trainium2
NeuronCores, data-parallel over the batch (each core owns B/8 = 2048 samples
and a full replica of the embedding tables; no collectives needed).

Device algorithm per core (feature-major activations: [features_p, samples_f]):
  1. bottom MLP 13->512->256->64 via lhsT=W matmuls (out = W.T @ x), bf16
  2. embedding lookup: one InstDMAGatherAnt per table (2048 int16 indices of
     512B 4-row units), then a DVE predicated-merge selects the right 64-dim
     row out of each unit (row-in-unit masks are host-computed, free-dim only)
  3. HWDGE DMA-transposes (split across the sync+scalar rings) turn gathered
     sample-major tiles into feature-major TM slabs [64h+d, 27, 128] with
     sample-chunk parity h stacked on partitions
  4. per-sample Gram Z_s = T_s @ T_s.T as K=64 M=N=27 bf16 matmuls, packed
     4-way across tile_position (64h, 32c); strided PSUM layout
  5. interaction-to-top-MLP folded as Zflat[729] @ W2full (symmetrized,
     half-weighted, zero-diagonal copy of Wt0[64:]) -- no tril extraction
  6. top MLP 415->512->256->1 + sigmoid, f32 output
"""

import sys

if "/opt/trn_rl_repo" not in sys.path:
    sys.path.insert(0, "/opt/trn_rl_repo")

import ml_dtypes
import numpy as np

import concourse.bass as bass
import concourse.mybir as mybir
import concourse.tile as tile
from concourse import bacc
from concourse.bass_utils import run_bass_kernel_spmd

F32 = mybir.dt.float32
BF16 = mybir.dt.bfloat16
I16 = mybir.dt.int16
U8 = mybir.dt.uint8
AF = mybir.ActivationFunctionType

N_CORES = 8
B_TOTAL = 16384
B = B_TOTAL // N_CORES  # 2048 samples per core
T = 26                  # embedding tables
V = 100000              # vocab per table
D = 64                  # embedding dim
NI = 27                 # interaction vectors per sample (26 tables + dense)
N_DENSE = 13
H1, H2 = 512, 256       # bottom MLP dims
T1, T2 = 512, 256       # top MLP dims
NU = V // 4             # 25000 gather units (512B = 4 vocab rows) per table
NBANK = 32              # gram PSUM banks: 16 u-slots x 27 i-stripes each

_DEBUG = False


def _build_nc():
    nc = bacc.Bacc(None, target_bir_lowering=False, num_swdge_queues=4)

    embg = nc.dram_tensor("embg", [T, NU, 256], BF16, kind="ExternalInput")
    gidx = nc.dram_tensor("gidx", [T, 128, B // 16], I16, kind="ExternalInput")
    gmask = nc.dram_tensor("gmask", [128, T, 3, 16], U8, kind="ExternalInput")
    dense_t = nc.dram_tensor("dense_t", [N_DENSE, B], BF16, kind="ExternalInput")
    wb0 = nc.dram_tensor("wb0", [N_DENSE, H1], BF16, kind="ExternalInput")
    bb0 = nc.dram_tensor("bb0", [128, H1 // 128], F32, kind="ExternalInput")
    wb1 = nc.dram_tensor("wb1", [H1, H2], BF16, kind="ExternalInput")
    bb1 = nc.dram_tensor("bb1", [128, H2 // 128], F32, kind="ExternalInput")
    wb2 = nc.dram_tensor("wb2", [H2, D], BF16, kind="ExternalInput")
    bb2 = nc.dram_tensor("bb2", [128, 1], F32, kind="ExternalInput")
    wt0d = nc.dram_tensor("wt0d", [D, T1], BF16, kind="ExternalInput")
    w2 = nc.dram_tensor("w2", [128, T1 // 128, NI * 128], BF16,
                        kind="ExternalInput")
    bt0 = nc.dram_tensor("bt0", [128, T1 // 128], F32, kind="ExternalInput")
    wt1 = nc.dram_tensor("wt1", [T1, T2], BF16, kind="ExternalInput")
    bt1 = nc.dram_tensor("bt1", [128, T2 // 128], F32, kind="ExternalInput")
    wt2 = nc.dram_tensor("wt2", [T2, 1], BF16, kind="ExternalInput")
    bt2 = nc.dram_tensor("bt2", [1, 1], F32, kind="ExternalInput")
    out = nc.dram_tensor("out", [1, B], F32, kind="ExternalOutput")
    if _DEBUG:
        dbg_d = nc.dram_tensor("dbg_d", [128, B], F32, kind="ExternalOutput")
        dbg_s = nc.dram_tensor("dbg_s", [128, NBANK * 432], F32,
                               kind="ExternalOutput")
        dbg_y1 = nc.dram_tensor("dbg_y1", [128, 4, B], F32, kind="ExternalOutput")
        dbg_tm = nc.dram_tensor("dbg_tm", [128, NI, 128], F32,
                                kind="ExternalOutput")

    with tile.TileContext(nc) as tc:
        with (
            tc.tile_pool(name="const", bufs=1) as cp,
            tc.tile_pool(name="mm", bufs=2, space="PSUM") as mmp,
            tc.tile_pool(name="tm", bufs=4) as tmp,
            tc.tile_pool(name="h", bufs=1) as hp,
            tc.tile_pool(name="big", bufs=1) as bigp,
            tc.tile_pool(name="w2p", bufs=2) as w2p,
        ):
            # ---- constants / weights ----
            wb0s = cp.tile([N_DENSE, H1], BF16)
            nc.sync.dma_start(wb0s[:], wb0[:])
            wb1s = cp.tile([128, 4, H2], BF16)
            for k in range(4):
                nc.sync.dma_start(wb1s[:, k, :], wb1[128 * k:128 * (k + 1), :])
            wb2s = cp.tile([128, 2, D], BF16)
            for k in range(2):
                nc.sync.dma_start(wb2s[:, k, :], wb2[128 * k:128 * (k + 1), :])
            wt0ds = cp.tile([D, T1], BF16)
            nc.sync.dma_start(wt0ds[:], wt0d[:])
            wt1s = cp.tile([128, 4, T2], BF16)
            for k in range(4):
                nc.sync.dma_start(wt1s[:, k, :], wt1[128 * k:128 * (k + 1), :])
            wt2s = cp.tile([128, 2, 1], BF16)
            for k in range(2):
                nc.sync.dma_start(wt2s[:, k, :], wt2[128 * k:128 * (k + 1), :])
            bb0s = cp.tile([128, H1 // 128], F32)
            nc.sync.dma_start(bb0s[:], bb0[:])
            bb1s = cp.tile([128, H2 // 128], F32)
            nc.sync.dma_start(bb1s[:], bb1[:])
            bb2s = cp.tile([128, 1], F32)
            nc.sync.dma_start(bb2s[:], bb2[:])
            bt0s = cp.tile([128, T1 // 128], F32)
            nc.sync.dma_start(bt0s[:], bt0[:])
            bt1s = cp.tile([128, T2 // 128], F32)
            nc.sync.dma_start(bt1s[:], bt1[:])
            bt2s = cp.tile([1, 1], F32)
            nc.sync.dma_start(bt2s[:], bt2[:])
            dts = cp.tile([N_DENSE, B], BF16)
            nc.sync.dma_start(dts[:], dense_t[:])
            masks = cp.tile([128, T, 3, 16], U8)
            nc.sync.dma_start(masks[:], gmask[:])
            idxs = cp.tile([128, T, B // 16], I16)
            for t in range(T):
                nc.sync.dma_start(idxs[:, t, :], gidx[t, :, :])

            # d^T replicated on both partition halves: D2[64h + d, s]
            D2 = cp.tile([128, B], BF16)

            # ---- phase 1: bottom MLP ----
            for n in range(4):
                sl = slice(512 * n, 512 * (n + 1))
                h1t = hp.tile([128, 4, 512], BF16, tag="h1")
                for m in range(4):
                    ps = mmp.tile([128, 512], F32)
                    nc.tensor.matmul(ps[:], wb0s[:, 128 * m:128 * (m + 1)],
                                     dts[:, sl], start=True, stop=True)
                    nc.scalar.activation(h1t[:, m, :], ps[:], AF.Relu,
                                         bias=bb0s[:, m:m + 1])
                h2t = hp.tile([128, 2, 512], BF16, tag="h2")
                for m in range(2):
                    ps = mmp.tile([128, 512], F32)
                    for k in range(4):
                        nc.tensor.matmul(ps[:], wb1s[:, k, 128 * m:128 * (m + 1)],
                                         h1t[:, k, :], start=(k == 0), stop=(k == 3))
                    nc.scalar.activation(h2t[:, m, :], ps[:], AF.Relu,
                                         bias=bb1s[:, m:m + 1])
                ps = mmp.tile([128, 512], F32)
                for half in range(2):   # write d to both partition halves
                    for k in range(2):
                        nc.tensor.matmul(ps[64 * half:64 * half + 64, :],
                                         wb2s[:, k, :], h2t[:, k, :],
                                         start=(k == 0), stop=(k == 1),
                                         tile_position=(0, 64 * half))
                nc.scalar.activation(D2[:, sl], ps[:], AF.Relu, bias=bb2s[:])

            # ---- phase 2: gather + merge (one dma_gather per table) ----
            fins = []
            with tc.tile_pool(name="gth", bufs=3) as gthp, \
                 tc.tile_pool(name="fin", bufs=1) as finp:
                for t in range(T):
                    gt = gthp.tile([128, B // 128, 256], BF16, tag="gt")
                    nc.gpsimd.dma_gather(gt[:], embg[t, :, :], idxs[:, t, :],
                                         B, B, 256, single_packet=False,
                                         queue_num=t % 4)
                    fin = finp.tile([128, B // 128, D], BF16, tag=f"fin{t}")
                    fins.append(fin)
                    nc.vector.tensor_copy(fin[:], gt[:, :, 0:D])
                    for r in (1, 2, 3):
                        nc.vector.copy_predicated(
                            fin[:],
                            masks[:, t, r - 1, :].to_broadcast([128, B // 128, D]),
                            gt[:, :, D * r:D * (r + 1)])

                # ---- phase 3: DMA-transposes + grams ----
                S = bigp.tile([128, NBANK * 432], BF16, tag="S")
                Sr = S[:].rearrange("p (bk i u) -> p bk i u", i=NI, u=16)
                with tc.tile_pool(name="gram", bufs=4, space="PSUM") as gramp:
                    for q in (0, 1):
                        tmts = {}
                        for ci in range(4):
                            pbt = 2 * ci + q
                            tmt = tmp.tile([128, NI, 128], BF16)
                            tmts[ci] = tmt
                            for t in range(T):
                                eng = nc.sync if t % 2 == 0 else nc.scalar
                                src = fins[t][:].rearrange("p b d -> p (b d)")
                                eng.dma_start_transpose(
                                    tmt[:, t, :],
                                    src[:, 128 * pbt:128 * pbt + 128])
                            for h in range(2):
                                s0 = 256 * pbt + 128 * h
                                nc.vector.tensor_copy(
                                    tmt[64 * h:64 * h + 64, T, :],
                                    D2[64 * h:64 * h + 64, s0:s0 + 128])
                            if _DEBUG and pbt == 0:
                                nc.gpsimd.dma_start(dbg_tm[:], tmt[:])
                        for bank16 in range(16):
                            bk = gramp.tile([128, NI, 16], F32)
                            h = bank16 // 8
                            for u16 in range(16):
                                j = (bank16 % 8) * 16 + u16
                                for c in range(4):
                                    slab = tmts[c][64 * h:64 * h + 64, :, j]
                                    nc.tensor.matmul(
                                        bk[32 * c:32 * c + NI, :, u16],
                                        slab, slab, start=True, stop=True,
                                        tile_position=(64 * h, 32 * c))
                            bank = 16 * q + bank16
                            nc.vector.tensor_copy(
                                S[:, 432 * bank:432 * (bank + 1)],
                                bk[:].rearrange("p i u -> p (i u)"))

            if _DEBUG:
                nc.gpsimd.dma_start(dbg_s[:], S[:])
                nc.gpsimd.dma_start(dbg_d[:], D2[:])

            # ---- phase 4: fold-in + top-MLP layer 1 ----
            y1 = bigp.tile([128, 4, B], BF16, tag="y1")
            with tc.tile_pool(name="fold", bufs=4, space="PSUM") as foldp:
                for m in range(4):
                    w2m = w2p.tile([128, NI * 128], BF16)
                    nc.sync.dma_start(w2m[:], w2[:, m, :])
                    w2mr = w2m[:].rearrange("p (i o) -> p i o", o=128)
                    yps = []
                    for c in range(4):
                        yp = foldp.tile([128, 512], F32)
                        yps.append(yp)
                        nc.tensor.matmul(
                            yp[:], wt0ds[:, 128 * m:128 * (m + 1)],
                            D2[0:D, 512 * c:512 * (c + 1)],
                            start=True, stop=False)
                    for i in range(NI):
                        for c in range(4):
                            nc.tensor.matmul(
                                yps[c][:], w2mr[32 * c:32 * c + NI, i, :],
                                Sr[32 * c:32 * c + NI, :, i, :],
                                start=False, stop=(i == NI - 1),
                                tile_position=(32 * c, 0))
                    for c in range(4):
                        nc.scalar.activation(y1[:, m, 512 * c:512 * (c + 1)],
                                             yps[c][:], AF.Relu,
                                             bias=bt0s[:, m:m + 1])

            if _DEBUG:
                nc.gpsimd.dma_start(dbg_y1[:], y1[:])

            # ---- phase 5: top-MLP layer 2 ----
            y2 = bigp.tile([128, 2, B], BF16, tag="y2")
            for n in range(4):
                for m in range(2):
                    ps = mmp.tile([128, 512], F32)
                    for k in range(4):
                        nc.tensor.matmul(ps[:], wt1s[:, k, 128 * m:128 * (m + 1)],
                                         y1[:, k, 512 * n:512 * (n + 1)],
                                         start=(k == 0), stop=(k == 3))
                    nc.scalar.activation(y2[:, m, 512 * n:512 * (n + 1)], ps[:],
                                         AF.Relu, bias=bt1s[:, m:m + 1])

            # ---- phase 6: top-MLP layer 3 + sigmoid ----
            outs = bigp.tile([1, B], F32, tag="outs")
            for n in range(4):
                ps = mmp.tile([128, 512], F32)
                for k in range(2):
                    nc.tensor.matmul(ps[0:1, :], wt2s[:, k, :],
                                     y2[:, k, 512 * n:512 * (n + 1)],
                                     start=(k == 0), stop=(k == 1))
                nc.scalar.activation(outs[0:1, 512 * n:512 * (n + 1)], ps[0:1, :],
                                     AF.Sigmoid, bias=bt2s[:, :])
            nc.sync.dma_start(out[:], outs[:])

    nc.finalize()
    return nc


_NC_CACHE = None


def _get_nc():
    global _NC_CACHE
    if _NC_CACHE is None:
        _NC_CACHE = _build_nc()
    return _NC_CACHE


def _rep_bias(b, parts=128):
    b = np.asarray(b, np.float32)
    if b.size < parts:
        assert parts % b.size == 0
        return np.tile(b, parts // b.size).reshape(parts, 1)
    return np.ascontiguousarray(b.reshape(-1, parts).T)


def _wrap16(x):
    """index list [B] -> ucode layout [128, B/16]: entry i at (i%16, i//16),
    replicated across the 8 Q7 core groups."""
    w = x.reshape(B // 16, 16).T
    return np.ascontiguousarray(np.tile(w, (8, 1)))


def _host_prep(dense, sparse_idx, emb, Wb0, bb0, Wb1, bb1, Wb2, bb2,
               Wt0, bt0, Wt1, bt1, Wt2, bt2):
    bf = ml_dtypes.bfloat16
    embg = np.ascontiguousarray(
        np.asarray(emb, np.float32).astype(bf).reshape(T, NU, 256))

    Wt0 = np.asarray(Wt0, np.float32)
    li, lj = np.tril_indices(NI, -1)
    W2full = np.zeros((NI, NI, T1), np.float32)
    W2full[li, lj] = 0.5 * Wt0[D:]
    W2full[lj, li] = 0.5 * Wt0[D:]
    W2r = W2full.reshape(NI, NI, 4, 128)          # i, j, m, o
    W2rep = np.zeros((128, 4, NI, 128), np.float32)
    for c in range(4):
        W2rep[32 * c:32 * c + NI] = W2r.transpose(1, 2, 0, 3)
    w2 = np.ascontiguousarray(W2rep.reshape(128, 4, NI * 128).astype(bf))

    shared = dict(
        embg=embg,
        wb0=np.asarray(Wb0, np.float32).astype(bf),
        bb0=_rep_bias(bb0),
        wb1=np.asarray(Wb1, np.float32).astype(bf),
        bb1=_rep_bias(bb1),
        wb2=np.asarray(Wb2, np.float32).astype(bf),
        bb2=_rep_bias(bb2),
        wt0d=np.asarray(Wt0[:D], np.float32).astype(bf),
        w2=w2,
        bt0=_rep_bias(bt0),
        wt1=np.asarray(Wt1, np.float32).astype(bf),
        bt1=_rep_bias(bt1),
        wt2=np.asarray(Wt2, np.float32).astype(bf),
        bt2=np.asarray(bt2, np.float32).reshape(1, 1),
    )

    dense = np.asarray(dense, np.float32)
    idx = np.asarray(sparse_idx).astype(np.int64)
    in_maps = []
    for core in range(N_CORES):
        sl = slice(core * B, (core + 1) * B)
        ishard = idx[sl]                          # [B, 26]
        gi = np.zeros((T, 128, B // 16), np.int16)
        gm = np.zeros((128, T, 3, 16), np.uint8)
        for t in range(T):
            it = ishard[:, t]
            gi[t] = _wrap16((it // 4).astype(np.int16))
            r = (it % 4).astype(np.int64)         # row within 512B unit
            rt = r.reshape(B // 128, 128).T       # [128p, 16blk]
            for rr in (1, 2, 3):
                gm[:, t, rr - 1, :] = (rt == rr).astype(np.uint8)
        m = dict(shared)
        m["dense_t"] = np.ascontiguousarray(dense[sl].T.astype(bf))
        m["gidx"] = gi
        m["gmask"] = np.ascontiguousarray(gm)
        in_maps.append(m)
    return in_maps


def kernel(dense, sparse_idx, emb, Wb0, bb0, Wb1, bb1, Wb2, bb2,
           Wt0, bt0, Wt1, bt1, Wt2, bt2, _trace=False, _trace_kwargs=None):
    nc = _get_nc()
    in_maps = _host_prep(dense, sparse_idx, emb, Wb0, bb0, Wb1, bb1, Wb2, bb2,
                         Wt0, bt0, Wt1, bt1, Wt2, bt2)
    res = run_bass_kernel_spmd(nc, in_maps, core_ids=list(range(N_CORES)),
                               trace=_trace, **(_trace_kwargs or {}))
    outp = np.concatenate([res.results[c]["out"].reshape(-1)
                           for c in range(N_CORES)])
    if _trace:
        kernel._last_results = res
    return outp

